# revision 19
# baseline (speedup 1.0000x reference)
"""Trainium2 Bass kernel for fused MHA block (QKV -> masked softmax attention
-> out-proj -> residual -> LayerNorm), sharded over 8 NeuronCores.

Sharding: core c handles batch b=c//4 and query rows [512*r, 512*(r+1)) with
r=c%4. Each core computes QKV for its own 512 rows, AllGathers K^T and V
across the 4 cores of its batch (bf16, 4 chunks so the collective overlaps
attention), runs attention for its rows over all 16 heads (scores computed
transposed [k, q] so no on-chip transposes are ever needed), then
out-projection + residual + LayerNorm natively.

v6 structure:
 - all matmul operands bf16 (fp32 accumulate in PSUM).
 - QKV weights prefetched whole into resident SBUF tiles at t=0 on the
   gpsimd queue; big attention/epilogue operand loads are deferred behind
   the collective doorbells (DMA descriptor issue time scales with size and
   serializes per queue).
 - mask applied multiplicatively on the Vector engine after exp (keep-mask
   in {0,1}); the PE runs only scores / AV / denominator matmuls.
 - AV packs 2 heads per matmul slot via column tiling; softmax denominators
   are computed in separate 64-row column tiles, so the reciprocal runs on
   all 128 lanes and multiplies straight into the attnT buffer.
 - K/V gathered chunks land in per-k-tile resident SBUF tiles.
 - AV/denominator matmuls lag scores/exp/mask by one k-tile (software
   pipeline) so the PE never blocks inline on the scalar engine.
"""

import numpy as np
import ml_dtypes

from concourse import bacc, bass_utils, mybir, tile
import concourse.bass as bass

B, S, D = 2, 2048, 1024
H, DH = 16, 64
SL = 512  # per-core query-row shard
NCORES = 8
R = 4  # ranks per replica group (one batch)
GROUPS = [[0, 1, 2, 3], [4, 5, 6, 7]]

f32 = mybir.dt.float32
bf16 = mybir.dt.bfloat16
AF = mybir.ActivationFunctionType
ALU = mybir.AluOpType


def _build():
    nc = bacc.Bacc("TRN2", target_bir_lowering=False, debug=False,
                   num_devices=NCORES)

    xT = nc.dram_tensor("xT", [D, SL], bf16, kind="ExternalInput")
    wqkv = nc.dram_tensor("wqkv", [D, 3 * D], bf16, kind="ExternalInput")
    bq = nc.dram_tensor("bq", [128, 8], f32, kind="ExternalInput")
    bk = nc.dram_tensor("bk", [128, 8], f32, kind="ExternalInput")
    bv = nc.dram_tensor("bv", [1, D], f32, kind="ExternalInput")
    wout = nc.dram_tensor("wout", [D, D], bf16, kind="ExternalInput")
    keepT = nc.dram_tensor("keepT", [S, SL], bf16, kind="ExternalInput")
    xres = nc.dram_tensor("xres", [SL, D], f32, kind="ExternalInput")
    lng = nc.dram_tensor("lng", [1, D], f32, kind="ExternalInput")
    lnb = nc.dram_tensor("lnb", [1, D], f32, kind="ExternalInput")
    out = nc.dram_tensor("out", [SL, D], f32, kind="ExternalOutput")

    with tile.TileContext(nc) as tc:
        _body(tc, nc, xT, wqkv, bq, bk, bv, wout, keepT, xres, lng, lnb, out)
    nc.compile()
    return nc


def _body(tc, nc, xT, wqkv, bq, bk, bv, wout, keepT, xres, lng, lnb, out):
    with (
        tc.tile_pool(name="singles", bufs=1) as singles,
        tc.tile_pool(name="dpool", bufs=1, space="DRAM") as dpool,
    ):
        # ---- constants / long-lived tiles ----
        ones64 = singles.tile([128, 64], bf16)
        nc.vector.memset(ones64, 1.0)
        bqs = singles.tile([128, 8], f32)
        nc.sync.dma_start(out=bqs, in_=bq.ap())
        bks = singles.tile([128, 8], f32)
        nc.sync.dma_start(out=bks, in_=bk.ap())
        bvb = singles.tile([128, D], f32)
        nc.sync.dma_start(out=bvb, in_=bv.ap().to_broadcast([128, D]))
        lngb = singles.tile([128, D], f32)
        lnbb = singles.tile([128, D], f32)
        epss = singles.tile([128, 1], f32)
        nc.vector.memset(epss, 1e-5)
        # preload the exp table set during phase 1
        dummye = singles.tile([128, 1], bf16)
        nc.scalar.activation(out=dummye, in_=epss, func=AF.Exp, scale=1.0)
        # keep-mask slabs, duplicated over the head-pair dim so the DVE
        # multiply sees a contiguous [128, 2, 512] operand per k-tile.
        keep2_sb = singles.tile([128, 16, 2, SL], bf16)
        xres_sb = singles.tile([128, 4, D], f32)
        qT_t = [singles.tile([128, SL], bf16, name=f"qTt{pt}", tag=f"qTt{pt}")
                for pt in range(8)]
        attnT_sb = singles.tile([128, 8, SL], bf16)
        y_sb = singles.tile([128, 4, D], f32)

        # collective bounce buffers: per k-chunk c (128 local k rows),
        # block 0 = kT cols chunk [1024, 128], block 1 = v rows chunk
        # [128, 1024], both flattened to 131072 elements. bf16.
        CH = D * 128
        kv_loc = [dpool.tile([2, 2, CH], bf16, name=f"kvloc{g}")
                  for g in range(2)]
        kv_ag = [dpool.tile([R, 2, 2, CH], bf16, name=f"kvag{g}")
                 for g in range(2)]

        # ---- phase 1: QKV projection for this core's 512 rows ----
        with (
            tc.tile_pool(name="qkv_w", bufs=1) as wpool,
            tc.tile_pool(name="qkv_sb", bufs=1) as qsb,
            tc.tile_pool(name="qkv_ps", bufs=8, space="PSUM") as qps,
        ):
            xT_sb = qsb.tile([128, 8, SL], bf16)
            nc.scalar.dma_start(
                out=xT_sb, in_=xT.ap().rearrange("(t p) q -> p t q", p=128)
            )
            kT_sb = qsb.tile([128, 8, SL], bf16)
            v_sb = qsb.tile([128, 4, D], bf16)

            # resident weight tiles for all of wqkv, issued on gpsimd at t=0
            kw, vw, qw = [], [], []
            for kt in range(8):
                w = wpool.tile([128, D], bf16, name=f"kw{kt}", tag=f"kw{kt}")
                nc.gpsimd.dma_start(
                    out=w, in_=wqkv.ap()[kt * 128:(kt + 1) * 128, D:2 * D])
                kw.append(w)
                w = wpool.tile([128, D], bf16, name=f"vw{kt}", tag=f"vw{kt}")
                nc.gpsimd.dma_start(
                    out=w, in_=wqkv.ap()[kt * 128:(kt + 1) * 128, 2 * D:3 * D])
                vw.append(w)
            for kt in range(8):
                w = wpool.tile([128, D], bf16, name=f"qw{kt}", tag=f"qw{kt}")
                nc.gpsimd.dma_start(
                    out=w, in_=wqkv.ap()[kt * 128:(kt + 1) * 128, 0:D])
                qw.append(w)

            # --- K^T: out tiles pt=0..8 cover W cols [1024, 2048) ---
            ps_k = [qps.tile([128, SL], f32, name=f"psk{pt}", tag="qkvps")
                    for pt in range(8)]
            for pt in range(8):
                for kt in range(8):
                    nc.tensor.matmul(
                        ps_k[pt], (kw[kt][:, pt * 128:(pt + 1) * 128]),
                        (xT_sb[:, kt, :]), start=(kt == 0), stop=(kt == 7),
                    )
                nc.vector.tensor_scalar(
                    out=kT_sb[:, pt, :], in0=ps_k[pt],
                    scalar1=bks[:, pt:pt + 1], scalar2=None, op0=ALU.add,
                )
            for c in range(4):
                # kT cols chunk c: [1024, 128] row-major == (t p f) flat
                nc.sync.dma_start(
                    out=kv_loc[c // 2][c % 2, 0, :]
                    .rearrange("(t p f) -> p t f", p=128, f=128),
                    in_=kT_sb[:, :, c * 128:(c + 1) * 128],
                )

            # --- V chunk-wise; AllGather chunk c fires as soon as its V
            # rows chunk lands (K chunks are already written) ---
            for c in range(4):
                ps_vc = [qps.tile([128, SL], f32, name=f"psv{c}_{i}",
                                  tag="qkvps") for i in range(2)]
                for kt in range(8):
                    for nch in range(2):
                        nc.tensor.matmul(
                            ps_vc[nch],
                            (xT_sb[:, kt, c * 128:(c + 1) * 128]),
                            (vw[kt][:, nch * 512:(nch + 1) * 512]),
                            start=(kt == 0), stop=(kt == 7),
                        )
                for nch in range(2):
                    nc.vector.tensor_add(
                        out=v_sb[:, c, nch * 512:(nch + 1) * 512],
                        in0=ps_vc[nch],
                        in1=bvb[:, nch * 512:(nch + 1) * 512],
                    )
                nc.sync.dma_start(
                    out=kv_loc[c // 2][c % 2, 1, :]
                    .rearrange("(p f) -> p f", p=128),
                    in_=v_sb[:, c, :],
                )
                if c % 2 == 1:
                    nc.gpsimd.collective_compute(
                        "AllGather", ALU.bypass, replica_groups=GROUPS,
                        ins=[kv_loc[c // 2].opt()],
                        outs=[kv_ag[c // 2].opt()],
                    )

            # deferred big input loads: attention/epilogue operands, issued
            # behind the collective doorbells so they never delay weights.
            for i in range(2):
                nc.sync.dma_start(
                    out=keep2_sb[:, :, i, :],
                    in_=keepT.ap().rearrange("(t p) q -> p t q", p=128),
                )
            nc.sync.dma_start(
                out=xres_sb, in_=xres.ap().rearrange("(t p) d -> p t d",
                                                     p=128))
            if not LN_IDENT:
                nc.sync.dma_start(out=lngb,
                                  in_=lng.ap().to_broadcast([128, D]))
                nc.sync.dma_start(out=lnbb,
                                  in_=lnb.ap().to_broadcast([128, D]))

            # --- Q^T: out tiles pt=0..8 cover W cols [0, 1024) ---
            ps_q = [qps.tile([128, SL], f32, name=f"psq{pt}", tag="qkvps")
                    for pt in range(8)]
            for pt in range(8):
                for kt in range(8):
                    nc.tensor.matmul(
                        ps_q[pt], (qw[kt][:, pt * 128:(pt + 1) * 128]),
                        (xT_sb[:, kt, :]), start=(kt == 0), stop=(kt == 7),
                    )
                nc.vector.tensor_scalar(
                    out=qT_t[pt], in0=ps_q[pt],
                    scalar1=bqs[:, pt:pt + 1], scalar2=None, op0=ALU.add,
                )

        # ---- phase 2: attention (scores transposed [k, q]) ----
        # k-tile kt = 4*j + c holds global k rows [512*j + 128*c, +128).
        with (
            tc.tile_pool(name="att_kres", bufs=1) as krp,
            tc.tile_pool(name="att_vres", bufs=1) as vrp,
            tc.tile_pool(name="att_pr", bufs=6) as prp,
            tc.tile_pool(name="att_pm", bufs=6) as pmp,
            tc.tile_pool(name="att_rd", bufs=2) as rdp,
            tc.tile_pool(name="att_ps", bufs=2, space="PSUM") as psp,
            tc.tile_pool(name="att_av", bufs=2, space="PSUM") as avp,
            tc.tile_pool(name="att_dn", bufs=2, space="PSUM") as dnp,
        ):
            # resident K^T / V tiles, one per k-tile: deps stay per-AG-chunk
            kres = [None] * 16
            vres = [None] * 16
            for c in range(4):
                for j in range(4):
                    kt = 4 * j + c
                    kres[kt] = krp.tile([128, 8, 128], bf16, name=f"kres{kt}",
                                        tag=f"kres{kt}")
                    nc.sync.dma_start(
                        out=kres[kt],
                        in_=kv_ag[c // 2][j, c % 2, 0, :]
                        .rearrange("(t p f) -> p t f", p=128, f=128),
                    )
                    vres[kt] = vrp.tile([128, 1024], bf16, name=f"vres{kt}",
                                        tag=f"vres{kt}")
                    nc.sync.dma_start(
                        out=vres[kt],
                        in_=kv_ag[c // 2][j, c % 2, 1, :]
                        .rearrange("(p f) -> p f", p=128),
                    )

            for hg in range(4):
                av = [avp.tile([128, 512], f32, name=f"av{hg}_{hp}", tag="av")
                      for hp in range(2)]
                dn = [dnp.tile([128, 512], f32, name=f"dn{hg}_{hp}", tag="dn")
                      for hp in range(2)]
                prms = [None] * 16  # masked-prob tiles, indexed by it

                def emit_scores(it, kt):
                    prms[it] = []
                    for hp in range(2):
                        ps = psp.tile([128, 2, 512], f32,
                                      name=f"ps{hg}_{kt}_{hp}", tag="ps")
                        for i in range(2):
                            hh = 2 * hp + i
                            h = 4 * hg + hh
                            po = (hh % 2) * 64
                            nc.tensor.matmul(
                                ps[:, i, :],
                                (kres[kt][po:po + 64, h // 2, :]),
                                (qT_t[h // 2][po:po + 64, :]),
                                start=True, stop=True,
                            )
                        pr = prp.tile([128, 2, 512], bf16,
                                      name=f"pr{hg}_{kt}_{hp}", tag="pr")
                        nc.scalar.activation(out=pr, in_=ps, func=AF.Exp,
                                             scale=0.125)
                        prm = pmp.tile([128, 2, 512], bf16,
                                       name=f"pm{hg}_{kt}_{hp}", tag="pm")
                        nc.vector.tensor_mul(
                            out=prm, in0=pr, in1=keep2_sb[:, kt, :, :]
                        )
                        prms[it].append(prm)

                def emit_av(it, kt):
                    first, last = (it == 0), (it == 15)
                    for hp in range(2):
                        prm = prms[it][hp]
                        for i in range(2):
                            h = 4 * hg + 2 * hp + i
                            nc.tensor.matmul(
                                av[hp][i * 64:(i + 1) * 64, :],
                                (vres[kt][:, h * 64:(h + 1) * 64]),
                                (prm[:, i, :]), start=first, stop=last,
                            )
                        for i in range(2):
                            nc.tensor.matmul(
                                dn[hp][i * 64:(i + 1) * 64, :],
                                (ones64), (prm[:, i, :]),
                                start=first, stop=last,
                            )

                kts = [4 * j + c for c in range(4) for j in range(4)]
                for it, kt in enumerate(kts):
                    emit_scores(it, kt)
                    if it > 0:
                        emit_av(it - 1, kts[it - 1])
                emit_av(15, kts[15])

                # normalize: all-lane reciprocal of the 64-row denominator
                # blocks, multiply straight into attnT.
                for hp in range(2):
                    rd = rdp.tile([128, 512], f32, name=f"rd{hg}_{hp}",
                                  tag="rd")
                    nc.vector.reciprocal(out=rd, in_=dn[hp])
                    nc.vector.tensor_mul(
                        out=attnT_sb[:, 2 * hg + hp, :],
                        in0=av[hp], in1=rd,
                    )

        # ---- phase 3: out-projection + residual + LayerNorm ----
        with (
            tc.tile_pool(name="op_w", bufs=1) as wop,
            tc.tile_pool(name="op_ps", bufs=8, space="PSUM") as opps,
            tc.tile_pool(name="ln", bufs=4) as lnp,
        ):
            # prefetch all wout tiles during attention
            wot = []
            for nch in range(2):
                for kt in range(8):
                    w = wop.tile([128, 512], bf16, name=f"wot{nch}_{kt}",
                                 tag=f"wot{nch}_{kt}")
                    nc.gpsimd.dma_start(
                        out=w,
                        in_=wout.ap()[kt * 128:(kt + 1) * 128,
                                      nch * 512:(nch + 1) * 512],
                    )
                    wot.append(w)
            for qt in range(4):
                yps = [opps.tile([128, 512], f32, name=f"yps{qt}_{nch}",
                                 tag="yps") for nch in range(2)]
                for kt in range(8):
                    for nch in range(2):
                        nc.tensor.matmul(
                            yps[nch],
                            (attnT_sb[:, kt, qt * 128:(qt + 1) * 128]),
                            (wot[nch * 8 + kt]), start=(kt == 0), stop=(kt == 7),
                        )
                for nch in range(2):
                    nc.vector.tensor_add(
                        out=y_sb[:, qt, nch * 512:(nch + 1) * 512],
                        in0=yps[nch],
                        in1=xres_sb[:, qt, nch * 512:(nch + 1) * 512],
                    )
                stats = lnp.tile([128, 2, 6], f32, name=f"st{qt}", tag="st")
                for i in range(2):
                    nc.vector.bn_stats(
                        out=stats[:, i, :], in_=y_sb[:, qt, i * 512:(i + 1) * 512]
                    )
                mv = lnp.tile([128, 2], f32, name=f"mv{qt}", tag="mv")
                nc.vector.bn_aggr(out=mv, in_=stats)
                nc.scalar.activation(
                    out=mv[:, 1:2], in_=mv[:, 1:2], func=AF.Sqrt,
                    bias=epss, scale=1.0,
                )
                nc.vector.reciprocal(out=mv[:, 1:2], in_=mv[:, 1:2])
                yt = lnp.tile([128, D], f32, name=f"yt{qt}", tag="yt")
                nc.vector.tensor_scalar(
                    out=yt, in0=y_sb[:, qt, :], scalar1=mv[:, 0:1],
                    scalar2=mv[:, 1:2], op0=ALU.subtract, op1=ALU.mult,
                )
                if not LN_IDENT:
                    nc.vector.tensor_mul(out=yt, in0=yt, in1=lngb)
                    nc.vector.tensor_add(out=yt, in0=yt, in1=lnbb)
                nc.sync.dma_start(
                    out=out.ap()[qt * 128:(qt + 1) * 128, :], in_=yt
                )


_NC_CACHE = None
LN_IDENT = False


def kernel(**inputs) -> np.ndarray:
    x = np.ascontiguousarray(np.asarray(inputs["x"], dtype=np.float32))
    W_attn = np.ascontiguousarray(np.asarray(inputs["W_attn"], np.float32))
    b_attn = np.asarray(inputs["b_attn"], np.float32)
    W_out = np.ascontiguousarray(np.asarray(inputs["W_out"], np.float32))
    b_out = np.asarray(inputs["b_out"], np.float32)
    ln_g = np.asarray(inputs["ln_g"], np.float32)
    ln_b = np.asarray(inputs["ln_b"], np.float32)
    mask = np.asarray(inputs["mask"])

    global LN_IDENT, _NC_CACHE
    want_ident = bool(np.all(ln_g == 1.0) and np.all(ln_b == 0.0))
    if _NC_CACHE is None or want_ident != LN_IDENT:
        LN_IDENT = want_ident
        _NC_CACHE = _build()
    nc = _NC_CACHE

    bqa = np.ascontiguousarray(b_attn[0:D].reshape(8, 128).T)
    bka = np.ascontiguousarray(b_attn[D:2 * D].reshape(8, 128).T)
    bva = np.ascontiguousarray(b_attn[2 * D:3 * D].reshape(1, D))
    wqkv_bf = np.ascontiguousarray(W_attn.astype(ml_dtypes.bfloat16))
    wout_bf = np.ascontiguousarray(W_out.astype(ml_dtypes.bfloat16))
    in_maps = []
    for c in range(NCORES):
        b, r = divmod(c, R)
        rows = slice(SL * r, SL * (r + 1))
        xTl = np.ascontiguousarray(x[b, rows, :].T.astype(ml_dtypes.bfloat16))
        keepTl = np.ascontiguousarray(
            (~mask[b, 0, rows, :]).T.astype(ml_dtypes.bfloat16))
        xresl = np.ascontiguousarray(x[b, rows, :] + b_out[None, :])
        in_maps.append(dict(
            xT=xTl, wqkv=wqkv_bf, bq=bqa, bk=bka, bv=bva, wout=wout_bf,
            keepT=keepTl, xres=xresl, lng=ln_g.reshape(1, D),
            lnb=ln_b.reshape(1, D),
        ))

    res = bass_utils.run_bass_kernel_spmd(nc, in_maps,
                                          core_ids=list(range(NCORES)))
    kernel.last_results = res

    full = np.empty((B, S, D), np.float32)
    for c in range(NCORES):
        b, r = divmod(c, R)
        full[b, SL * r:SL * (r + 1), :] = res.results[c]["out"]
    return full


if __name__ == "__main__":
    rng = np.random.default_rng(0)
    ins = dict(
        x=rng.standard_normal((B, S, D), dtype=np.float32),
        W_attn=rng.standard_normal((D, 3 * D), dtype=np.float32) / 32,
        b_attn=np.zeros(3 * D, np.float32),
        W_out=rng.standard_normal((D, D), dtype=np.float32) / 32,
        b_out=np.zeros(D, np.float32),
        ln_g=np.ones(D, np.float32),
        ln_b=np.zeros(D, np.float32),
        mask=rng.integers(0, 5, (B, 1, S, S)) == 0,
    )
    y = kernel(**ins)
    print("ok", y.shape, y.dtype)
